# revision 7
# baseline (speedup 1.0000x reference)
"""Trainium2 Bass kernel: CTC segment-mean compression (segment_reduce).

Reference semantics (per batch element b):
  pred = argmax_V(logit)                  # softmax skipped: argmax-invariant
  segments = run-length groups of pred; padded frames excluded
  out[s, b, :] = mean of representation frames in segment s (0 if s unused)
  padding_out[b, s] = s >= num_segments(b)

Sharding: data-parallel over batch B=16 across 8 NeuronCores (2 each),
zero communication. Each core streams its 82MB logit shard (memory-bound).
"""

import numpy as np

import concourse.bass as bass
import concourse.tile as tile
from concourse import bacc, mybir
from concourse.bass_utils import run_bass_kernel_spmd

T, B, D, V = 1024, 16, 512, 10000
NCORES = 8
BL = B // NCORES          # batch elems per core = 2
NT = T // 128             # t-tiles per batch elem = 8
C = BL * NT               # (b, ttile) rows = 16
BIG = 4096.0              # out-of-range segment id offset for padded frames

# If True, the per-frame max over V runs on GPSIMD (Pool) so the DVE only
# does max_index; otherwise DVE does both passes.
MAX_ON_GPSIMD = False

f32 = mybir.dt.float32
bf16 = mybir.dt.bfloat16
u8 = mybir.dt.uint8
u32 = mybir.dt.uint32
OP = mybir.AluOpType
AX = mybir.AxisListType


def _build_nc():
    nc = bacc.Bacc()

    rep_ext = nc.declare_dram_parameter("representation", [T, BL, D], f32, isOutput=False)
    logit_ext = nc.declare_dram_parameter("logit", [T, BL, V], f32, isOutput=False)
    pad_ext = nc.declare_dram_parameter("padding", [BL, T], u8, isOutput=False)
    ident_ext = nc.declare_dram_parameter("c_ident", [128, 128], f32, isOutput=False)
    iota_ext = nc.declare_dram_parameter("c_iota", [128, T], f32, isOutput=False)
    shift_ext = nc.declare_dram_parameter("c_shift", [C, C], f32, isOutput=False)
    lexc_ext = nc.declare_dram_parameter("c_lexc", [C, C], f32, isOutput=False)
    k0_ext = nc.declare_dram_parameter("c_k0", [C, 1], f32, isOutput=False)
    out_ext = nc.declare_dram_parameter("out", [T, BL, D], f32, isOutput=True)
    pout_ext = nc.declare_dram_parameter("pad_out", [BL, T], u8, isOutput=True)

    with tile.TileContext(nc) as tc:
        with (
            tc.tile_pool(name="const", bufs=1) as constp,
            tc.tile_pool(name="logit", bufs=3 if MAX_ON_GPSIMD else 2) as logitp,
            tc.tile_pool(name="mx", bufs=4) as mxp,
            tc.tile_pool(name="pred", bufs=1) as predp,
            tc.tile_pool(name="seg", bufs=1) as segp,
            tc.tile_pool(name="rep", bufs=2) as repp,
            tc.tile_pool(name="w", bufs=NT + 1) as wp,
            tc.tile_pool(name="eout", bufs=3) as eoutp,
            tc.tile_pool(name="psA", bufs=2, space="PSUM") as psA,
            tc.tile_pool(name="psB", bufs=2, space="PSUM") as psB,
        ):
            ident = constp.tile([128, 128], f32)
            nc.sync.dma_start(ident[:], ident_ext[:])
            iota = constp.tile([128, T], f32)
            nc.sync.dma_start(iota[:], iota_ext[:])
            shiftm = constp.tile([C, C], f32)
            nc.sync.dma_start(shiftm[:], shift_ext[:])
            lexcm = constp.tile([C, C], f32)
            nc.sync.dma_start(lexcm[:], lexc_ext[:])
            k0m = constp.tile([C, 1], f32)
            nc.sync.dma_start(k0m[:], k0_ext[:])
            ones_bf = constp.tile([128, 1], bf16)
            nc.vector.memset(ones_bf[:], 1.0)

            # ---- Stage 0: load representation early (few deps -> few waits) ----
            repbs = []
            for b in range(BL):
                rep_f = repp.tile([128, NT * D], f32, tag="repf")
                nc.sync.dma_start(
                    rep_f[:].rearrange("p (k d) -> p k d", k=NT),
                    rep_ext[:, b, :].rearrange("(k p) d -> p k d", p=128))
                repb = repp.tile([128, NT * D], bf16, tag="repb")
                nc.scalar.copy(repb[:], rep_f[:])
                repbs.append(repb)

            # ---- Stage 1: per-frame argmax over V ----
            pred_cols = predp.tile([128, C], f32)
            for b in range(BL):
                for k in range(NT):
                    c = b * NT + k
                    lg = logitp.tile([128, V], f32, tag="lg")
                    nc.sync.dma_start(lg[:], logit_ext[k * 128:(k + 1) * 128, b, :])
                    mx8 = mxp.tile([128, 8], f32, tag="mx8")
                    idx = mxp.tile([128, 8], u32, tag="idx")
                    if MAX_ON_GPSIMD:
                        nc.gpsimd.reduce_max(mx8[:, 0:1], lg[:], axis=AX.X)
                        nc.vector.max_index(idx[:], mx8[:, 0:1].to_broadcast((128, 8)), lg[:])
                    else:
                        nc.vector.max(mx8[:], lg[:])
                        nc.vector.max_index(idx[:], mx8[:], lg[:])
                    nc.vector.tensor_copy(pred_cols[:, c:c + 1], idx[:, 0:1])

            # ---- Stage 2: transpose pred -> [C, 128] (row c=(b,k), col t') ----
            ps_predT = psB.tile([C, 128], f32, tag="small")
            nc.tensor.matmul(ps_predT[:], lhsT=pred_cols[:], rhs=ident[:], start=True, stop=True)
            predT = segp.tile([C, 128], f32)
            nc.scalar.copy(predT[:], ps_predT[:])

            # ---- Stage 3: previous-frame boundary value per row ----
            # bprev[c] = predT[c-1, 127] (garbage for k==0 rows; masked below)
            ps_b = psB.tile([C, 1], f32, tag="small")
            nc.tensor.matmul(ps_b[:], lhsT=shiftm[:], rhs=predT[:, 127:128], start=True, stop=True)
            bprev = segp.tile([C, 1], f32)
            nc.scalar.copy(bprev[:], ps_b[:])

            # ---- Stage 4: change indicators ----
            change = segp.tile([C, 128], f32)
            nc.vector.scalar_tensor_tensor(
                change[:, 0:1], in0=predT[:, 0:1], scalar=bprev[:], in1=k0m[:],
                op0=OP.not_equal, op1=OP.mult)
            nc.vector.scalar_tensor_tensor(
                change[:, 1:128], in0=predT[:, 1:128], scalar=0.0, in1=predT[:, 0:127],
                op0=OP.add, op1=OP.not_equal)

            # ---- Stage 5: within-row inclusive cumsum ----
            segl = segp.tile([C, 128], f32)
            nc.vector.tensor_tensor_scan(
                segl[:], data0=change[:], data1=change[:], initial=0.0,
                op0=OP.add, op1=OP.bypass)

            # ---- Stage 6: cross-row carry (exclusive prefix within each b) ----
            ps_c = psB.tile([C, 1], f32, tag="small")
            nc.tensor.matmul(ps_c[:], lhsT=lexcm[:], rhs=segl[:, 127:128], start=True, stop=True)
            carry = segp.tile([C, 1], f32)
            nc.scalar.copy(carry[:], ps_c[:])
            seg0 = segp.tile([C, 128], f32)
            nc.vector.tensor_scalar(seg0[:], segl[:], carry[:], None, op0=OP.add)

            # ---- Stage 7: padding mask; out-of-range ids for padded frames ----
            pad_u8 = segp.tile([C, 128], u8)
            nc.sync.dma_start(pad_u8[:], pad_ext.rearrange("b (k t) -> (b k) t", k=NT))
            padf = segp.tile([C, 128], f32)
            nc.vector.tensor_copy(padf[:], pad_u8[:])
            masked = segp.tile([C, 128], f32)
            nc.vector.scalar_tensor_tensor(
                masked[:], in0=padf[:], scalar=-BIG, in1=seg0[:], op0=OP.mult, op1=OP.add)
            segsel = segp.tile([C, 128], f32)
            nc.vector.scalar_tensor_tensor(
                segsel[:], in0=padf[:], scalar=BIG, in1=seg0[:], op0=OP.mult, op1=OP.add)

            # ---- Stage 8: new_lengths -> padding_out ----
            rowmax = segp.tile([C, 1], f32)
            nc.vector.reduce_max(rowmax[:], masked[:], axis=AX.X)
            ps_rm = psB.tile([1, C], f32, tag="small")
            nc.tensor.matmul(ps_rm[:], lhsT=rowmax[:], rhs=ident[0:C, 0:C], start=True, stop=True)
            rm_row = segp.tile([1, C], f32)
            nc.scalar.copy(rm_row[:], ps_rm[:])
            nl_row = segp.tile([1, BL], f32)   # max valid seg id per b (= new_len - 1)
            nc.vector.reduce_max(nl_row[:], rm_row[:].rearrange("p (b k) -> p b k", k=NT), axis=AX.X)
            nl2 = segp.tile([BL, 1], f32)
            nc.sync.dma_start(nl2[:], nl_row[:])
            po = segp.tile([BL, T], u8)
            nc.vector.tensor_scalar(po[:], iota[0:BL, :], nl2[:], None, op0=OP.is_gt)
            nc.sync.dma_start(pout_ext[:], po[:])

            # ---- Stage 9: transpose segsel back to [128, C] ----
            ps_segT = psB.tile([128, C], f32, tag="segT")
            nc.tensor.matmul(ps_segT[:], lhsT=segsel[:], rhs=ident[0:C, 0:C], start=True, stop=True)
            segT = segp.tile([128, C], f32)
            nc.scalar.copy(segT[:], ps_segT[:])

            # ---- Stage 10: segment mean via one-hot matmul ----
            for b in range(BL):
                repb = repbs[b]
                ws = []
                for k in range(NT):
                    c = b * NT + k
                    w = wp.tile([128, T], bf16, tag="wk")
                    nc.vector.tensor_scalar(w[:], iota[:], segT[:, c:c + 1], None, op0=OP.is_equal)
                    ws.append(w)
                for m in range(NT):
                    ps_out = psA.tile([128, D], f32, tag="big")
                    ps_cnt = psB.tile([128, 1], f32, tag="cnt")
                    for k in range(NT):
                        nc.tensor.matmul(
                            ps_out[:], lhsT=ws[k][:, m * 128:(m + 1) * 128],
                            rhs=repb[:, k * D:(k + 1) * D],
                            start=(k == 0), stop=(k == NT - 1))
                    for k in range(NT):
                        nc.tensor.matmul(
                            ps_cnt[:], lhsT=ws[k][:, m * 128:(m + 1) * 128],
                            rhs=ones_bf[:],
                            start=(k == 0), stop=(k == NT - 1))
                    cnt_cl = eoutp.tile([128, 1], f32, tag="cnt_cl")
                    nc.vector.tensor_scalar_max(cnt_cl[:], ps_cnt[:], 1.0)
                    rcp = eoutp.tile([128, 1], f32, tag="rcp")
                    nc.vector.reciprocal(rcp[:], cnt_cl[:])
                    osb = eoutp.tile([128, D], f32, tag="osb")
                    nc.scalar.activation(
                        osb[:], ps_out[:], mybir.ActivationFunctionType.Copy, scale=rcp[:])
                    nc.sync.dma_start(out_ext[m * 128:(m + 1) * 128, b, :], osb[:])

    nc.finalize()
    return nc


_NC_CACHE = None


def _consts():
    ident = np.eye(128, dtype=np.float32)
    iota = np.tile(np.arange(T, dtype=np.float32)[None, :], (128, 1))
    shift = np.eye(C, C, 1, dtype=np.float32)          # shift[k, m] = (m == k+1)
    kk = np.arange(C)
    lexc = ((kk[:, None] // NT == kk[None, :] // NT) & (kk[:, None] < kk[None, :])
            ).astype(np.float32)
    k0 = (kk % NT != 0).astype(np.float32)[:, None]
    return {
        "c_ident": ident, "c_iota": np.ascontiguousarray(iota),
        "c_shift": shift, "c_lexc": lexc, "c_k0": np.ascontiguousarray(k0),
    }


def _run(representation, logit, padding, trace=False):
    global _NC_CACHE
    if _NC_CACHE is None:
        _NC_CACHE = _build_nc()
    nc = _NC_CACHE

    rep = np.ascontiguousarray(np.asarray(representation, dtype=np.float32))
    lg = np.ascontiguousarray(np.asarray(logit, dtype=np.float32))
    pad = np.ascontiguousarray(np.asarray(padding).astype(np.uint8))
    consts = _consts()

    in_maps = []
    for i in range(NCORES):
        b0 = i * BL
        m = {
            "representation": np.ascontiguousarray(rep[:, b0:b0 + BL, :]),
            "logit": np.ascontiguousarray(lg[:, b0:b0 + BL, :]),
            "padding": np.ascontiguousarray(pad[b0:b0 + BL, :]),
        }
        m.update(consts)
        in_maps.append(m)

    res = run_bass_kernel_spmd(nc, in_maps, list(range(NCORES)), trace=trace)
    out = np.concatenate([res.results[i]["out"] for i in range(NCORES)], axis=1)
    pout = np.concatenate([res.results[i]["pad_out"] for i in range(NCORES)], axis=0)
    return (out, pout.astype(bool)), res


def kernel(representation, logit, padding):
    (out, pout), _ = _run(representation, logit, padding, trace=False)
    return out, pout
